# revision 21
# baseline (speedup 1.0000x reference)
"""
DeepAttMISL segment-reduce kernel for Trainium2 (Bass/Tile), 8 NeuronCores.

Math (see reference):
  h        = relu(x @ W1.T + b1)                    x:[N,1024] -> h:[N,256]
  seg      = segment_sum(h, cluster_id, 8)          -> [8,256]
  h_clust  = seg / max(counts,1)
  h_path   = relu(h_clust @ Wf.T + bf)
  A        = softmax((tanh(h_path@Wa.T+ba) * sigmoid(h_path@Wb.T+bb)) @ Wc.T + bc)
  H        = A @ h_path                             -> [1,256]

Sharding: rows (instances) N=65536 split across 8 cores (8192 each).
Each core computes h for its shard and accumulates the per-cluster
segment sums DIRECTLY in transposed layout segT[hid, k] (h-tile
stationary, one-hot moving), so no PE transposes are needed before the
head.  The 8KB partial is AllReduced (ncfw); a dummy collective at
t~0 pre-warms ncfw so the real one starts fast.  (A p2p
remote_dma_broadcast exchange was tried and is architecturally better,
but the SWDGE desc-gen instruction faults under this axon/fake-NRT
runtime.)  Every core redundantly computes the tiny attention head;
core 0's output is returned (host reshapes [128,2] -> [1,256]).

Precision: big matmul in bf16 (inputs rounded once on host) with fp32
PSUM accumulation; everything downstream fp32.  sigmoid(y) is computed
as 0.5*(1+tanh(y/2)) (0.5 folded into Wb/bb/Wc on host) so the whole
kernel only needs relu/tanh/exp -- all in one ACT table set, no
mid-kernel table reloads.  bc is dropped (softmax shift-invariant).
"""

import sys

if "/opt/trn_rl_repo" not in sys.path:
    sys.path.insert(0, "/opt/trn_rl_repo")

import numpy as np
import ml_dtypes

import concourse.bass as bass
import concourse.tile as tile
from concourse import bacc, mybir
from concourse import bass_utils

ALU = mybir.AluOpType

N_CORES = 8
N_TOTAL = 65536
N_SHARD = N_TOTAL // N_CORES          # 8192 rows per core
DIN = 1024
DHID = 256
K_CL = 8                               # clusters
KC = DIN // 128                        # 8 contraction chunks
ROWT = N_SHARD // 128                  # 64 row-tiles of 128 rows
SB_SIZES = [768, 1280, 1536, 1536, 1536, 1536]   # x superblocks (ramped head)
assert sum(SB_SIZES) == N_SHARD
SEG_DELAY = 3                          # row-tiles between h and its seg MMs

# head-const blob layout (f32 elements per partition)
OFF_WFT = 0                            # Wf.T tiled   [2,256] -> 512
OFF_WAT = 512                          # Wa.T tiled   [2,256] -> 512
OFF_WBT = 1024                         # (Wb/2).T     [2,256] -> 512
OFF_WCR = 1536                         # (Wc/2) bcast [2,128] -> 256
OFF_BFC = 1792                         # bf           [2]
OFF_BAC = 1794                         # ba           [2]
OFF_BBC = 1796                         # bb/2         [2]
OFF_INV = 1798                         # 1/count tiled x2 [16]
NBLOB = 1814

BF16 = mybir.dt.bfloat16
F32 = mybir.dt.float32
AF = mybir.ActivationFunctionType

_CACHE = {}


def _build_nc():
    nc = bacc.Bacc("TRN2", target_bir_lowering=False, debug=False,
                   num_devices=N_CORES)

    xT = nc.dram_tensor("xT", [DIN, N_SHARD], BF16, kind="ExternalInput")
    w1t = nc.dram_tensor("w1t", [DIN, DHID], BF16, kind="ExternalInput")
    moh = nc.dram_tensor("moh", [128, ROWT, K_CL], BF16, kind="ExternalInput")
    b1b = nc.dram_tensor("b1b", [128, DHID], F32, kind="ExternalInput")
    blob = nc.dram_tensor("blob", [128, NBLOB], F32, kind="ExternalInput")

    out = nc.dram_tensor("out", [128, 2], F32, kind="ExternalOutput")

    with tile.TileContext(nc) as tc:
        with tc.tile_pool(name="consts", bufs=1) as consts, \
             tc.tile_pool(name="xblk", bufs=3) as xblk, \
             tc.tile_pool(name="hpool", bufs=6) as hpool, \
             tc.tile_pool(name="hps", bufs=4, space="PSUM") as hps, \
             tc.tile_pool(name="segps", bufs=1, space="PSUM") as segps, \
             tc.tile_pool(name="headps", bufs=2, space="PSUM") as headps, \
             tc.tile_pool(name="small", bufs=1) as small, \
             tc.tile_pool(name="dram", bufs=1, space="DRAM") as dram:

            # ---- dummy collective at t~0: wakes/warms ncfw so the real
            # AllReduce's trigger->start latency (~11us observed cold) is
            # paid here, overlapped with the main loop.  Nothing reads its
            # output, so no core blocks on it.
            warm_z = small.tile([128, 1], F32)
            nc.vector.memset(warm_z[:], 0.0)
            wcc_in = dram.tile([128, 1], F32)
            wcc_out = dram.tile([128, 1], F32)
            nc.scalar.dma_start(wcc_in[:], warm_z[:])
            nc.gpsimd.collective_compute(
                "AllReduce", ALU.add,
                replica_groups=[list(range(N_CORES))],
                ins=[wcc_in[:].opt()], outs=[wcc_out[:].opt()])

            # ---- PE warm-up: ~16 dummy matmuls on a zeroed tile keep the
            # PE busy through the initial DMA preload so HAM un-throttles
            # (2.4GHz) before real work, instead of ramping mid-loop.
            wz = consts.tile([128, DHID], BF16)
            nc.vector.memset(wz[:], 0.0)
            wps = hps.tile([128, DHID], F32, tag="hp")
            for _ in range(16):
                nc.tensor.matmul(wps[:], wz[:, 0:128], wz[:],
                                 start=True, stop=True, skip_group_check=True)

            # ---- critical consts: W1.T split across both HWDGE rings ----
            w1t_sb = consts.tile([128, KC, DHID], BF16)
            w1v = w1t.ap().rearrange("(k p) f -> p k f", p=128)
            nc.sync.dma_start(w1t_sb[:, 0:4, :], w1v[:, 0:4, :])
            nc.scalar.dma_start(w1t_sb[:, 4:8, :], w1v[:, 4:8, :])

            # ---- x superblock tiles; sb0's DMAs right behind W1 ----
            xts_blocks = []
            for sb, sbr in enumerate(SB_SIZES):
                xts = xblk.tile([128, KC, sbr], BF16, tag="xts",
                                padded_shape=[128, KC, max(SB_SIZES)],
                                name=f"xts{sb}")
                xts_blocks.append(xts)
            for k in range(KC):
                eng = nc.sync if k % 2 == 0 else nc.scalar
                eng.dma_start(xts_blocks[0][:, k, :],
                              xT.ap()[k * 128:(k + 1) * 128, 0:SB_SIZES[0]])

            # bias / one-hot right after sb0 on each ring
            b1b_sb = consts.tile([128, DHID], F32)
            nc.sync.dma_start(b1b_sb[:], b1b.ap())
            m_sb = consts.tile([128, ROWT, K_CL], BF16)
            nc.scalar.dma_start(m_sb[:], moh.ap())

            # ---- transposed segment-sum accumulators (live whole loop).
            # One PSUM bank PER group: a matmul's start=True clears the
            # whole bank's has_written bits, so two interleaved accumulation
            # groups in one bank lose the first group's opening tile.
            segT = [segps.tile([128, K_CL], F32, padded_shape=[128, 128],
                               name=f"segT{j}") for j in range(2)]

            def emit_seg(ph, pt):
                for j in range(2):
                    nc.tensor.matmul(
                        segT[j][:],
                        ph[:, j * 128:(j + 1) * 128], m_sb[:, pt, :],
                        start=(pt == 0), stop=(pt == ROWT - 1),
                        skip_group_check=True)

            # ---- main loop ----
            pending = []
            row0 = 0
            for sb, sbr in enumerate(SB_SIZES):
                xts = xts_blocks[sb]
                if sb > 0:
                    for k in range(KC):
                        eng = nc.sync if k % 2 == 0 else nc.scalar
                        eng.dma_start(
                            xts[:, k, :],
                            xT.ap()[k * 128:(k + 1) * 128, row0:row0 + sbr])
                for tl in range(sbr // 128):
                    t = row0 // 128 + tl
                    hp = hps.tile([128, DHID], F32, tag="hp")
                    for k in range(KC):
                        nc.tensor.matmul(
                            hp[:],
                            xts[:, k, tl * 128:(tl + 1) * 128],
                            w1t_sb[:, k, :],
                            start=(k == 0), stop=(k == KC - 1),
                            skip_group_check=True)
                    nc.vector.tensor_add(hp[:], hp[:], b1b_sb[:])
                    h_sb = hpool.tile([128, DHID], BF16)
                    nc.scalar.activation(h_sb[:], hp[:], AF.Relu)
                    pending.append((h_sb, t))
                    if len(pending) > SEG_DELAY:
                        ph, pt = pending.pop(0)
                        emit_seg(ph, pt)
                row0 += sbr
            while pending:
                ph, pt = pending.pop(0)
                emit_seg(ph, pt)

            # head consts: issued after ALL x DMAs on the sync ring so they
            # never delay the x stream (ring is FIFO); land ~15us before use
            blob_sb = consts.tile([128, NBLOB], F32)
            nc.sync.dma_start(blob_sb[:], blob.ap())

            # ---- AllGather the 8KB transposed partials (fewer ncfw mesh
            # rounds than AllReduce); each core sums the 8 on DVE ----
            seg_loc = small.tile([128, 2 * K_CL], F32)
            nc.vector.tensor_copy(seg_loc[:, 0:K_CL], segT[0][:])
            nc.vector.tensor_copy(seg_loc[:, K_CL:2 * K_CL], segT[1][:])
            ag_in = dram.tile([128, 2 * K_CL], F32)
            ag_out = dram.tile([N_CORES * 128, 2 * K_CL], F32)
            nc.sync.dma_start(ag_in[:], seg_loc[:])
            nc.gpsimd.collective_compute(
                "AllGather", ALU.bypass,
                replica_groups=[list(range(N_CORES))],
                ins=[ag_in[:].opt()], outs=[ag_out[:].opt()])
            rx = small.tile([128, N_CORES, 2 * K_CL], F32)
            nc.sync.dma_start(
                rx[:], ag_out[:].rearrange("(c p) f -> p c f", p=128))
            tot = small.tile([128, 2 * K_CL], F32)
            nc.vector.tensor_add(tot[:], rx[:, 0, :], rx[:, 1, :])
            for c in range(2, N_CORES):
                nc.vector.tensor_add(tot[:], tot[:], rx[:, c, :])

            # ---- cluster means (1/count varies along free dim) ----
            hcT = small.tile([128, 2 * K_CL], F32)
            nc.vector.tensor_mul(hcT[:], tot[:],
                                 blob_sb[:, OFF_INV:OFF_INV + 16])

            # ---- attention head, transposed layout [hid(2x128), k] ----
            def head_mm(w_off, rhs, b_off, func, name):
                o = small.tile([128, 2 * K_CL], F32, name=name)
                for j in range(2):
                    ps = headps.tile([128, K_CL], F32, tag="head",
                                     padded_shape=[128, 128])
                    for i in range(2):
                        nc.tensor.matmul(
                            ps[:],
                            blob_sb[:, w_off + i * 256 + j * 128:
                                    w_off + i * 256 + (j + 1) * 128],
                            rhs[:, i * K_CL:(i + 1) * K_CL],
                            start=(i == 0), stop=(i == 1))
                    nc.scalar.activation(o[:, j * K_CL:(j + 1) * K_CL], ps[:],
                                         func,
                                         bias=blob_sb[:, b_off + j:b_off + j + 1])
                return o

            hpT = head_mm(OFF_WFT, hcT, OFF_BFC, AF.Relu, "hpT")
            aT = head_mm(OFF_WAT, hpT, OFF_BAC, AF.Tanh, "aT")
            tT = head_mm(OFF_WBT, hpT, OFF_BBC, AF.Tanh, "tT")
            # a*g = a*0.5*(1+tanh) ; the 0.5 lives in Wc/2
            ag = small.tile([128, 2 * K_CL], F32)
            nc.vector.tensor_mul(ag[:], aT[:], tT[:])
            nc.vector.tensor_add(ag[:], ag[:], aT[:])

            # logits replicated on all 128 partitions (bc dropped: softmax
            # is shift-invariant)
            a_ps = headps.tile([128, K_CL], F32, tag="head",
                               padded_shape=[128, 128])
            for j in range(2):
                nc.tensor.matmul(
                    a_ps[:],
                    blob_sb[:, OFF_WCR + j * 128:OFF_WCR + (j + 1) * 128],
                    ag[:, j * K_CL:(j + 1) * K_CL],
                    start=(j == 0), stop=(j == 1))

            # softmax over 8 clusters (bounded logits; skip max-shift);
            # exp+rowsum fused via accum_out
            ea = small.tile([128, K_CL], F32)
            ssum = small.tile([128, 1], F32)
            nc.scalar.activation(ea[:], a_ps[:], AF.Exp, accum_out=ssum[:])
            rs = small.tile([128, 1], F32)
            nc.vector.reciprocal(rs[:], ssum[:])
            an = small.tile([128, K_CL], F32)
            nc.vector.tensor_scalar_mul(an[:], ea[:], rs[:, 0:1])

            # H[hid] = sum_k A[k] * h_path.T[hid, k]
            h_out = small.tile([128, 2], F32)
            for j in range(2):
                tmp = small.tile([128, K_CL], F32, name=f"wtmp{j}")
                nc.vector.tensor_mul(tmp[:], hpT[:, j * K_CL:(j + 1) * K_CL],
                                     an[:])
                nc.vector.reduce_sum(h_out[:, j:j + 1], tmp[:],
                                     axis=mybir.AxisListType.X)
            nc.sync.dma_start(out.ap()[:, :], h_out[:])

    nc.compile()
    return nc


def _prep_inputs(x_path, cluster_id, W1, b1, Wf, bf, Wa, ba, Wb, bb, Wc, bc):
    """Host-side sharding / marshalling. Returns in_maps for the 8 cores."""
    x = np.asarray(x_path, dtype=np.float32).reshape(N_TOTAL, DIN)
    cid = np.asarray(cluster_id).astype(np.int64).reshape(N_TOTAL)

    xb = x.astype(ml_dtypes.bfloat16)

    # one-hot cluster matrix, pre-tiled to [128, ROWT, K] per core
    oh = (cid[:, None] == np.arange(K_CL)[None, :]).astype(ml_dtypes.bfloat16)

    counts = np.bincount(cid, minlength=K_CL).astype(np.float32)
    invc = (1.0 / np.maximum(counts, 1.0)).astype(np.float32)

    W1 = np.asarray(W1, np.float32); b1 = np.asarray(b1, np.float32)
    Wf = np.asarray(Wf, np.float32); bf = np.asarray(bf, np.float32)
    Wa = np.asarray(Wa, np.float32); ba = np.asarray(ba, np.float32)
    Wb = np.asarray(Wb, np.float32); bb = np.asarray(bb, np.float32)
    Wc = np.asarray(Wc, np.float32)

    def tiled_T(M):  # [256,256] -> [128, 2, 256]; [p,i,f] = M.T[i*128+p, f]
        return np.ascontiguousarray(M.T.reshape(2, 128, DHID).transpose(1, 0, 2))

    blob = np.zeros((128, NBLOB), np.float32)
    blob[:, OFF_WFT:OFF_WFT + 512] = tiled_T(Wf).reshape(128, 512)
    blob[:, OFF_WAT:OFF_WAT + 512] = tiled_T(Wa).reshape(128, 512)
    blob[:, OFF_WBT:OFF_WBT + 512] = tiled_T(Wb * 0.5).reshape(128, 512)
    # (Wc/2) broadcast: [q, j*128+c] = Wc[0, j*128+q]/2 for all c
    wcr = np.broadcast_to((Wc.ravel() * 0.5).reshape(2, 128, 1),
                          (2, 128, 128)).transpose(1, 0, 2)
    blob[:, OFF_WCR:OFF_WCR + 256] = wcr.reshape(128, 256)
    blob[:, OFF_BFC:OFF_BFC + 2] = bf.reshape(2, 128).T
    blob[:, OFF_BAC:OFF_BAC + 2] = ba.reshape(2, 128).T
    blob[:, OFF_BBC:OFF_BBC + 2] = (bb * 0.5).reshape(2, 128).T
    blob[:, OFF_INV:OFF_INV + 16] = np.tile(invc, 2)[None, :]

    const_map = {
        "w1t": np.ascontiguousarray(W1.T).astype(ml_dtypes.bfloat16),
        "b1b": np.ascontiguousarray(np.broadcast_to(b1, (128, DHID))),
        "blob": blob,
    }

    in_maps = []
    for c in range(N_CORES):
        lo, hi = c * N_SHARD, (c + 1) * N_SHARD
        xT_c = np.ascontiguousarray(xb[lo:hi].T)            # [1024, 8192] bf16
        moh_c = np.ascontiguousarray(
            oh[lo:hi].reshape(ROWT, 128, K_CL).transpose(1, 0, 2))
        in_maps.append({"xT": xT_c, "moh": moh_c, **const_map})
    return in_maps


def kernel(**inputs):
    if "nc" not in _CACHE:
        _CACHE["nc"] = _build_nc()
    nc = _CACHE["nc"]
    in_maps = _prep_inputs(**inputs)
    res = bass_utils.run_bass_kernel_spmd(
        nc, in_maps, core_ids=list(range(N_CORES)))
    o = res.results[0]["out"]                               # [128, 2]
    return np.ascontiguousarray(o.T.reshape(1, DHID)).astype(np.float32)
